# revision 1
# baseline (speedup 1.0000x reference)
"""Multi-head attention (B=4, S=2048, D=1024, H=16) on 8 TRN2 NeuronCores.

Sharding: core c -> (batch b = c//2, head-group g = c%2). Each core computes
8 heads for one batch: QKV projections restricted to its 512 output dims,
attention for its heads, and a partial output projection over its 512
contraction dims of W_o. Host sums the two partial outputs per batch.

Device layout (per core, all matmul operands bf16, PSUM fp32):
  inputs  xqT/xkT/xvT [1024, S]   (X^T: feature dim on partitions)
          wqT/wkT/wvT [1024, 512] (W.T slices; 1/sqrt(dk) folded into wqT)
          woT         [512, 1024] (W_o slice transposed)
  output  outT        [1024, S]   (partial final^T, fp32)

Pipeline: Q^T/K^T projections (out-dim on partitions), V projection (seq on
partitions) with a ones column appended per head; per head: scores^T =
(K_h^T)^T-stationary x Q_h^T-moving so k-positions land on partitions; exp on
ScalarE directly from PSUM; PV with the ones column producing the softmax
denominator Z in column 64; per-partition reciprocal + scale on VectorE; PE
transpose of the normalized head output; final projection vs woT.
"""

import numpy as np
import ml_dtypes

B = 4
S = 2048
D = 1024
H_LOCAL = 8          # heads per core
DK = 64
G = H_LOCAL * DK     # 512 output dims per core
N_CORES = 8

_BUILD_CACHE = {}
_BUILD_VERSION = 5   # bump on any device-program change: busts the neuronxcc
                     # cache, whose module hash ignores custom_call contents

bf16 = ml_dtypes.bfloat16


def _build(s=S, debug=False, stage=5, delay_us=0):
    """stage: 1=DMA only, 2=+QKV proj, 3=+QK/exp, 4=+PV/drain, 5=full.
    delay_us: adds a nop chain of that length on SyncE (timing calibration)."""
    import concourse.tile as tile
    from concourse import bacc, mybir
    from concourse.masks import make_identity

    f32 = mybir.dt.float32
    b16 = mybir.dt.bfloat16

    assert s % 512 == 0
    NKT = s // 128          # k-position tiles per head
    NQT = s // 128          # q tiles per head
    NCH = s // 512          # 512-wide chunks of the seq dim
    HALF = min(s, 1024)     # QK psum tile width (<= 2 PSUM banks)
    NH = s // HALF          # halves per seq dim
    NCH_H = HALF // 512     # 512-chunks per half

    nc = bacc.Bacc("TRN2", target_bir_lowering=False, debug=False,
                   num_devices=N_CORES)

    xqT = nc.dram_tensor("xqT", [D, s], b16, kind="ExternalInput")
    xkT = nc.dram_tensor("xkT", [D, s], b16, kind="ExternalInput")
    xvT = nc.dram_tensor("xvT", [D, s], b16, kind="ExternalInput")
    wqT = nc.dram_tensor("wqT", [D, G], b16, kind="ExternalInput")
    wkT = nc.dram_tensor("wkT", [D, G], b16, kind="ExternalInput")
    wvT = nc.dram_tensor("wvT", [D, G], b16, kind="ExternalInput")
    woT = nc.dram_tensor("woT", [G, D], b16, kind="ExternalInput")
    nc.dram_tensor("vtag", [stage, _BUILD_VERSION + delay_us], f32,
                   kind="ExternalInput")
    outT = nc.dram_tensor("outT", [D, s], f32, kind="ExternalOutput")
    if debug:
        qTd = nc.dram_tensor("qTd", [G, s], b16, kind="ExternalOutput")
        kTd = nc.dram_tensor("kTd", [G, s], b16, kind="ExternalOutput")
        vd = nc.dram_tensor("vd", [s, H_LOCAL * (DK + 1)], b16,
                            kind="ExternalOutput")
        attnd = nc.dram_tensor("attnd", [s, s], b16, kind="ExternalOutput")
        aTd = nc.dram_tensor("aTd", [G, s], b16, kind="ExternalOutput")

    with tile.TileContext(nc) as tc:
        with (
            tc.tile_pool(name="w", bufs=1) as wpool,
            tc.tile_pool(name="big", bufs=36) as big,
            tc.tile_pool(name="vp", bufs=NKT) as vpool,
            tc.tile_pool(name="sm", bufs=4) as small,
            tc.tile_pool(name="ps", bufs=1, space="PSUM") as psum,
        ):
            # ---- weights + identity ----
            wq_s = wpool.tile([128, 8, G], b16, tag="wq")
            wk_s = wpool.tile([128, 8, G], b16, tag="wk")
            wv_s = wpool.tile([128, 8, G], b16, tag="wv")
            for t in range(8):
                nc.sync.dma_start(wq_s[:, t, :], wqT[t * 128:(t + 1) * 128, :])
                nc.sync.dma_start(wk_s[:, t, :], wkT[t * 128:(t + 1) * 128, :])
                nc.sync.dma_start(wv_s[:, t, :], wvT[t * 128:(t + 1) * 128, :])
            ident = wpool.tile([128, 128], b16, tag="ident")
            make_identity(nc, ident[:])


            # ablation plumbing: tiny live reads that defeat DCE per stage
            sink_t = wpool.tile([128, 512], f32, tag="sink")
            sink_n = [0]

            def sink(ap):
                c = sink_n[0]
                sink_n[0] += 1
                while len(ap.shape) > 2:
                    ap = ap[:, 0]
                nc.vector.tensor_copy(sink_t[0:1, c:c + 1], ap[0:1, 0:1])

            # ---- X^T inputs ----
            xq_s, xk_s, xv_s = [], [], []
            for src, dst in ((xqT, xq_s), (xkT, xk_s), (xvT, xv_s)):
                for t in range(8):
                    xt = big.tile([128, s], b16, tag="big")
                    nc.sync.dma_start(xt[:], src[t * 128:(t + 1) * 128, :])
                    dst.append(xt)

            if stage == 1:
                for xt in xq_s + xk_s + xv_s:
                    sink(xt)
                for wt in (wq_s, wk_s, wv_s):
                    sink(wt)

            # ---- Q^T / K^T projections: out [G, s], out-dim on partitions ----
            def proj_T(w_s, x_s, out_tiles, o):
                ot = big.tile([128, s], b16, tag="big")
                for half in range(NH):
                    ps = psum.tile([128, HALF], f32, tag="mm", bufs=2)
                    for c in range(NCH_H):
                        cs = slice(half * HALF + c * 512,
                                   half * HALF + (c + 1) * 512)
                        ls = slice(c * 512, (c + 1) * 512)
                        for i in range(8):
                            nc.tensor.matmul(
                                ps[:, ls],
                                lhsT=w_s[:, i, o * 128:(o + 1) * 128],
                                rhs=x_s[i][:, cs],
                                start=(i == 0), stop=(i == 7),
                            )
                    nc.vector.tensor_copy(
                        ot[:, half * HALF:(half + 1) * HALF], ps[:])
                out_tiles.append(ot)

            qT_s, kT_s = [], []

            # ---- V projection (one seq-tile step; interleaved into head 0) ----
            vp_s = []

            def emit_vproj_step(r):
                vt = vpool.tile([128, H_LOCAL, DK + 1], b16, tag="vp")
                nc.vector.memset(vt[:], 1.0)
                ps = psum.tile([128, 512], f32, tag="pv", bufs=4)
                for i in range(8):
                    nc.tensor.matmul(
                        ps[:],
                        lhsT=xv_s[i][:, r * 128:(r + 1) * 128],
                        rhs=wv_s[:, i, :],
                        start=(i == 0), stop=(i == 7),
                    )
                nc.vector.tensor_copy(
                    vt[:, :, 0:DK],
                    ps[:].rearrange("p (h d) -> p h d", h=H_LOCAL),
                )
                if debug:
                    nc.sync.dma_start(
                        vd[r * 128:(r + 1) * 128, :],
                        vt[:].rearrange("p h d -> p (h d)"))
                vp_s.append(vt)

            # ---- attention ----
            aT_s = [big.tile([128, s], b16, tag="big", name=f"aT{i}")
                    for i in range(4)]

            def head_slices(h):
                return h // 2, slice((h % 2) * 64, (h % 2) * 64 + 64)

            attn_tiles = {}   # (h, kt) -> sbuf tile
            pv_ps = {}        # h -> list of PV psum tiles

            def emit_qk(h, kt):
                ti, prt = head_slices(h)
                at = big.tile([128, s], b16, tag="big")
                for half in range(NH):
                    ps = psum.tile([128, HALF], f32, tag="mm", bufs=2)
                    for c in range(NCH_H):
                        cs = slice(half * HALF + c * 512,
                                   half * HALF + (c + 1) * 512)
                        nc.tensor.matmul(
                            ps[:, c * 512:(c + 1) * 512],
                            lhsT=kT_s[ti][prt, kt * 128:(kt + 1) * 128],
                            rhs=qT_s[ti][prt, cs],
                            start=True, stop=True,
                        )
                    nc.scalar.activation(
                        at[:, half * HALF:(half + 1) * HALF], ps[:],
                        func=mybir.ActivationFunctionType.Exp)
                if debug and h == 0:
                    nc.sync.dma_start(attnd[kt * 128:(kt + 1) * 128, :], at[:])
                attn_tiles[(h, kt)] = at

            def emit_pv_step(h, qt):
                pv = psum.tile([128, DK + 1], f32, tag="pv", bufs=4,
                               name=f"pvps{h}_{qt}")
                pv_ps[(h, qt)] = pv
                for kt in range(NKT):
                    nc.tensor.matmul(
                        pv[:],
                        lhsT=attn_tiles[(h, kt)][:, qt * 128:(qt + 1) * 128],
                        rhs=vp_s[kt][:, h, :],
                        start=(kt == 0), stop=(kt == NKT - 1),
                        skip_group_check=True,
                    )
                if qt == NQT - 1:
                    for kt in range(NKT):
                        attn_tiles.pop((h, kt))

            def emit_pv_drain(h, qt):
                ti, prt = head_slices(h)
                pv = pv_ps.pop((h, qt))
                rz = small.tile([128, 1], f32, tag="rz")
                nc.vector.reciprocal(rz[:], pv[:, 64:65])
                a_t = small.tile([128, DK], b16, tag="a")
                nc.vector.tensor_scalar_mul(a_t[:], pv[:, 0:64], rz[:])
                tr = psum.tile([64, 128], b16, tag="pv", bufs=4, name="trp")
                nc.tensor.transpose(tr[:], a_t[:], ident[:])
                nc.vector.tensor_copy(
                    aT_s[ti][prt, qt * 128:(qt + 1) * 128], tr[:])

            if stage == 2:
                for o in range(4):
                    proj_T(wq_s, xq_s, qT_s, o)
                for o in range(4):
                    proj_T(wk_s, xk_s, kT_s, o)
                for r in range(NKT):
                    emit_vproj_step(r)
                for t in qT_s + kT_s + vp_s:
                    sink(t)
            elif stage >= 3:
                for o in range(4):
                    proj_T(wq_s, xq_s, qT_s, o)
                for o in range(4):
                    proj_T(wk_s, xk_s, kT_s, o)
                for h in range(H_LOCAL):
                    for kt in range(NKT):
                        emit_qk(h, kt)
                        if h == 0:
                            emit_vproj_step(kt)
                        if stage == 3:
                            sink(attn_tiles.pop((h, kt)))
                        elif h > 0:
                            emit_pv_step(h - 1, kt)
                            emit_pv_drain(h - 1, kt)
                if stage >= 4:
                    for qt in range(NQT):
                        emit_pv_step(H_LOCAL - 1, qt)
                        emit_pv_drain(H_LOCAL - 1, qt)
                if stage == 3:
                    for t in vp_s:
                        sink(t)
                if stage == 4:
                    for t in aT_s:
                        sink(t)
                if debug:
                    for o in range(4):
                        nc.sync.dma_start(qTd[o * 128:(o + 1) * 128, :],
                                          qT_s[o][:])
                        nc.sync.dma_start(kTd[o * 128:(o + 1) * 128, :],
                                          kT_s[o][:])

            if debug:
                for i in range(4):
                    nc.sync.dma_start(aTd[i * 128:(i + 1) * 128, :], aT_s[i][:])

            if stage < 5:
                fo = small.tile([128, 512], f32, tag="fout", bufs=4)
                nc.vector.tensor_copy(fo[:], sink_t[:])
                nc.sync.dma_start(outT[0:128, 0:512], fo[:])

            # ---- output projection: outT[o*128:, c*512:] = sum_i woT_i.T @ aT_i ----
            if stage >= 5:
                wo_a = big.tile([128, 2, D], b16, tag="big", name="wo_a")
                wo_b = big.tile([128, 2, D], b16, tag="big", name="wo_b")
                for t in range(2):
                    nc.sync.dma_start(wo_a[:, t, :], woT[t * 128:(t + 1) * 128, :])
                    nc.sync.dma_start(wo_b[:, t, :],
                                      woT[(2 + t) * 128:(3 + t) * 128, :])
                wo_v = [wo_a[:, 0, :], wo_a[:, 1, :], wo_b[:, 0, :], wo_b[:, 1, :]]
            for o in range(8 if stage >= 5 else 0):
                for c in range(NCH):
                    cs = slice(c * 512, (c + 1) * 512)
                    ps = psum.tile([128, 512], f32, tag="pv", bufs=4)
                    for i in range(4):
                        nc.tensor.matmul(
                            ps[:],
                            lhsT=wo_v[i][:, o * 128:(o + 1) * 128],
                            rhs=aT_s[i][:, cs],
                            start=(i == 0), stop=(i == 3),
                        )
                    fo = small.tile([128, 512], f32, tag="fout", bufs=4)
                    nc.vector.tensor_copy(fo[:], ps[:])
                    nc.sync.dma_start(outT[o * 128:(o + 1) * 128, cs], fo[:])

    nc.compile()
    return nc


def _host_prep(Q_in, K_in, V_in, W_q, W_k, W_v, W_o, s=S):
    """Build per-core input maps (host-side shard + transpose + bf16 cast)."""
    in_maps = []
    scale = 1.0 / np.sqrt(np.float32(DK))
    for c in range(N_CORES):
        b, g = divmod(c, 2)
        gs = slice(g * G, (g + 1) * G)
        m = {
            "xqT": np.ascontiguousarray(Q_in[b].T).astype(bf16),
            "xkT": np.ascontiguousarray(K_in[b].T).astype(bf16),
            "xvT": np.ascontiguousarray(V_in[b].T).astype(bf16),
            "wqT": np.ascontiguousarray((W_q[gs, :] * scale).T).astype(bf16),
            "wkT": np.ascontiguousarray(W_k[gs, :].T).astype(bf16),
            "wvT": np.ascontiguousarray(W_v[gs, :].T).astype(bf16),
            "woT": np.ascontiguousarray(W_o[:, gs].T).astype(bf16),
            "vtag": np.zeros((5, _BUILD_VERSION), np.float32),
        }
        in_maps.append(m)
    return in_maps


def kernel(Q_in, K_in, V_in, W_q, W_k, W_v, W_o):
    from concourse.bass_utils import run_bass_kernel_spmd

    if "nc" not in _BUILD_CACHE:
        _BUILD_CACHE["nc"] = _build()
    nc = _BUILD_CACHE["nc"]

    in_maps = _host_prep(np.asarray(Q_in, np.float32), np.asarray(K_in, np.float32),
                         np.asarray(V_in, np.float32), np.asarray(W_q, np.float32),
                         np.asarray(W_k, np.float32), np.asarray(W_v, np.float32),
                         np.asarray(W_o, np.float32))
    res = run_bass_kernel_spmd(nc, in_maps, core_ids=list(range(N_CORES)))

    out = np.empty((B, S, D), np.float32)
    for b in range(B):
        acc = res.results[2 * b]["outT"] + res.results[2 * b + 1]["outT"]
        out[b] = acc.T
    return out



# revision 12
# speedup vs baseline: 167.0076x; 167.0076x over previous
"""Multi-head attention (B=4, S=2048, D=1024, H=16) on 8 TRN2 NeuronCores.

Sharding: core c -> (batch b = c//2, head-group g = c%2). Each core computes
8 heads for one batch: QKV projections restricted to its 512 output dims,
attention for its heads, and a partial output projection over its 512
contraction dims of W_o. Host sums the two partial outputs per batch.

v6 redesign vs the baseline:
  * QK: head pairs row-packed into the PE array (dk=64 contraction -> two
    concurrent 64-row tile_position matmuls), 2x QK throughput.
  * exp: alternates between ScalarE ACT-Exp and a DVE Schraudolph bit-trick
    exp (x*128/ln2 + bias -> int16 -> bitcast bf16), so softmax activation
    is no longer serialized on ScalarE alone.
  * PV: V-stationary dataflow (stationary = V head slice [128k, 65] incl. a
    ones column that accumulates the softmax denominator Z in psum row 64;
    moving = the [128k, 1024q] attn tile). No per-qt stationary reloads and
    no PE transposes: output lands directly as O^T [64d, q].
  * normalization: Z row -> DRAM -> [128,16] partition layout -> DVE
    reciprocal -> back -> GPSIMD partition_broadcast -> DVE multiply into
    the aT tiles consumed by the output projection.

Device layout (per core, matmul operands bf16, PSUM fp32):
  inputs  xqT/xkT/xvT [1024, S]   (X^T: feature dim on partitions)
          wqT/wkT/wvT [1024, 512] (W.T slices; 1/sqrt(dk) folded into wqT)
          woT         [512, 1024] (W_o slice transposed)
  output  outT        [1024, S]   (partial final^T, fp32)
"""

import numpy as np
import ml_dtypes

B = 4
S = 2048
D = 1024
H_LOCAL = 8          # heads per core
DK = 64
G = H_LOCAL * DK     # 512 output dims per core
N_CORES = 8

_BUILD_CACHE = {}
_BUILD_VERSION = 7   # bump on any device-program change: busts the neuronxcc
                     # cache, whose module hash ignores custom_call contents

bf16 = ml_dtypes.bfloat16

# Schraudolph exp in bf16: bits = int16(x * 128/ln2 + (127*128 - 7.3)),
# bitcast to bf16.  ~+-3% rel err; the mean multiplicative bias cancels in
# softmax (shared by numerator and denominator Z).
SCH_A = 184.66502678663007
SCH_B = 16248.7


def _build(s=S, debug=False, stage=5, delay_us=0, reps=1, dve_exp=True):
    """stage: 1=DMA only, 2=+QKV proj, 3=+QK/exp, 4/5=full.
    reps: wraps the body in a For_i hardware loop (timing builds); the
    body is idempotent so outputs are unchanged."""
    import contextlib
    import concourse.tile as tile
    from concourse import bacc, mybir

    f32 = mybir.dt.float32
    b16 = mybir.dt.bfloat16
    i16 = mybir.dt.int16

    assert s % 2048 == 0 or s in (512, 1024)
    NKT = s // 128          # k-position tiles per head
    QW = min(s, 1024)       # q-chunk width for the attention pipeline
    NQH = s // QW           # q chunks ("halves" at s=2048)
    HALF = min(s, 1024)     # projection psum tile width (<= 2 PSUM banks)
    NH = s // HALF
    NCH_H = HALF // 512
    NCH = s // 512

    nc = bacc.Bacc("TRN2", target_bir_lowering=False, debug=False,
                   num_devices=N_CORES)

    xqT = nc.dram_tensor("xqT", [D, s], b16, kind="ExternalInput")
    xkT = nc.dram_tensor("xkT", [D, s], b16, kind="ExternalInput")
    xvT = nc.dram_tensor("xvT", [D, s], b16, kind="ExternalInput")
    wqT = nc.dram_tensor("wqT", [D, G], b16, kind="ExternalInput")
    wkT = nc.dram_tensor("wkT", [D, G], b16, kind="ExternalInput")
    wvT = nc.dram_tensor("wvT", [D, G], b16, kind="ExternalInput")
    woT = nc.dram_tensor("woT", [G, D], b16, kind="ExternalInput")
    nc.dram_tensor("vtag", [stage + 8 * reps + (64 if dve_exp else 0),
                            _BUILD_VERSION + delay_us], f32,
                   kind="ExternalInput")
    outT = nc.dram_tensor("outT", [D, s], f32, kind="ExternalOutput")
    # DRAM scratch for the Z-row transpose dance (per pair x q-chunk)
    z_scr = [nc.dram_tensor(f"zscr{i}", [2, QW // 128, 128], f32,
                            kind="Internal") for i in range(4 * NQH)]
    rz_scr = [nc.dram_tensor(f"rzscr{i}", [2, QW // 128, 128], f32,
                             kind="Internal") for i in range(4 * NQH)]
    if debug:
        qTd = nc.dram_tensor("qTd", [G, s], b16, kind="ExternalOutput")
        kTd = nc.dram_tensor("kTd", [G, s], b16, kind="ExternalOutput")
        vd = nc.dram_tensor("vd", [s, H_LOCAL * (DK + 1)], b16,
                            kind="ExternalOutput")
        attnd = nc.dram_tensor("attnd", [s, s], b16, kind="ExternalOutput")
        aTd = nc.dram_tensor("aTd", [G, s], b16, kind="ExternalOutput")

    Exp = mybir.ActivationFunctionType.Exp
    MULT = mybir.AluOpType.mult
    ADD = mybir.AluOpType.add

    with tile.TileContext(nc) as tc:
      with (tc.For_i(0, reps) if reps > 1 else contextlib.nullcontext()):
        with (
            tc.tile_pool(name="w", bufs=1) as wpool,
            tc.tile_pool(name="big", bufs=32) as big,
            tc.tile_pool(name="vp", bufs=NKT) as vpool,
            tc.tile_pool(name="at", bufs=4) as atp,
            tc.tile_pool(name="dr", bufs=4) as dpool,
            tc.tile_pool(name="sm", bufs=4) as small,
        ):
            # ---- weights ----
            wq_s = wpool.tile([128, 8, G], b16, tag="wq")
            wk_s = wpool.tile([128, 8, G], b16, tag="wk")
            wv_s = wpool.tile([128, 8, G], b16, tag="wv")
            for t in range(8):
                nc.sync.dma_start(wq_s[:, t, :], wqT[t * 128:(t + 1) * 128, :])
                nc.sync.dma_start(wk_s[:, t, :], wkT[t * 128:(t + 1) * 128, :])
                nc.sync.dma_start(wv_s[:, t, :], wvT[t * 128:(t + 1) * 128, :])

            # ablation plumbing: tiny live reads that defeat DCE per stage
            sink_t = wpool.tile([128, 512], f32, tag="sink")
            sink_n = [0]

            def sink(ap):
                c = sink_n[0]
                sink_n[0] += 1
                while len(ap.shape) > 2:
                    ap = ap[:, 0]
                nc.vector.tensor_copy(sink_t[0:1, c:c + 1], ap[0:1, 0:1])

            # ---- X^T inputs ----
            xq_s, xk_s, xv_s = [], [], []
            for src, dst in ((xqT, xq_s), (xkT, xk_s), (xvT, xv_s)):
                for t in range(8):
                    xt = big.tile([128, s], b16, tag="big")
                    nc.sync.dma_start(xt[:], src[t * 128:(t + 1) * 128, :])
                    dst.append(xt)

            if stage == 1:
                for xt in xq_s + xk_s + xv_s:
                    sink(xt)
                for wt in (wq_s, wk_s, wv_s):
                    sink(wt)

            with tc.tile_pool(name="ps", bufs=1, space="PSUM") as psum:
                # ---- Q^T / K^T projections: [G, s], out-dim on partitions ----
                def proj_T(w_s, x_s, out_tiles, o):
                    ot = big.tile([128, s], b16, tag="big")
                    for half in range(NH):
                        ps = psum.tile([128, HALF], f32, tag="qk", bufs=2)
                        for c in range(NCH_H):
                            cs = slice(half * HALF + c * 512,
                                       half * HALF + (c + 1) * 512)
                            ls = slice(c * 512, (c + 1) * 512)
                            for i in range(8):
                                nc.tensor.matmul(
                                    ps[:, ls],
                                    lhsT=w_s[:, i, o * 128:(o + 1) * 128],
                                    rhs=x_s[i][:, cs],
                                    start=(i == 0), stop=(i == 7),
                                )
                        nc.vector.tensor_copy(
                            ot[:, half * HALF:(half + 1) * HALF], ps[:])
                    out_tiles.append(ot)

                qT_s, kT_s = [], []

                # ---- V projection: vp_s[kt] = [128 kpos, 8 heads, 65] ----
                vp_s = []

                def emit_vproj_group(base):
                    nsub = min(HALF // 512, NKT - base)
                    ps = psum.tile([128, HALF], f32, tag="qk", bufs=2)
                    for sub in range(nsub):
                        r = base + sub
                        hs = slice(sub * 512, (sub + 1) * 512)
                        for i in range(8):
                            nc.tensor.matmul(
                                ps[:, hs],
                                lhsT=xv_s[i][:, r * 128:(r + 1) * 128],
                                rhs=wv_s[:, i, :],
                                start=(i == 0), stop=(i == 7),
                            )
                    for sub in range(nsub):
                        r = base + sub
                        hs = slice(sub * 512, (sub + 1) * 512)
                        vt = vpool.tile([128, H_LOCAL, DK + 1], b16, tag="vp")
                        nc.vector.memset(vt[:], 1.0)
                        nc.vector.tensor_copy(
                            vt[:, :, 0:DK],
                            ps[:, hs].rearrange("p (h d) -> p h d",
                                                h=H_LOCAL),
                        )
                        if debug:
                            nc.sync.dma_start(
                                vd[r * 128:(r + 1) * 128, :],
                                vt[:].rearrange("p h d -> p (h d)"))
                        vp_s.append(vt)

                if stage >= 2:
                    for o in range(4):
                        proj_T(wq_s, xq_s, qT_s, o)
                    for o in range(4):
                        proj_T(wk_s, xk_s, kT_s, o)
                    for base in range(0, NKT, HALF // 512):
                        emit_vproj_group(base)

                if stage == 2:
                    for t in qT_s + kT_s + vp_s:
                        sink(t)

                # ---- attention: per head pair ti, per q-chunk qh ----
                aT_s = [big.tile([128, s], b16, tag="big", name=f"aT{i}")
                        for i in range(4)]

                def attn_pair(ti, qh):
                    qs = slice(qh * QW, (qh + 1) * QW)
                    pr_A = slice(0, 64)
                    pr_B = slice(64, 128)
                    hA, hB = 2 * ti, 2 * ti + 1
                    if stage >= 4:
                        pvA = psum.tile([65, QW], f32, tag="pv", bufs=2,
                                        name=f"pvA{ti}_{qh}")
                        pvB = psum.tile([65, QW], f32, tag="pv", bufs=2,
                                        name=f"pvB{ti}_{qh}")
                    at_prev = None
                    for kt in range(NKT + 1):
                        if kt < NKT:
                            ks = slice(kt * 128, (kt + 1) * 128)
                            psA = psum.tile([128, QW], f32, tag="qk", bufs=2)
                            psB = psum.tile([128, QW], f32, tag="qk", bufs=2)
                            for c2 in range(QW // 512):
                                c2s = slice(c2 * 512, (c2 + 1) * 512)
                                g2s = slice(qh * QW + c2 * 512,
                                            qh * QW + (c2 + 1) * 512)
                                nc.tensor.matmul(
                                    psA[:, c2s], lhsT=kT_s[ti][pr_A, ks],
                                    rhs=qT_s[ti][pr_A, g2s],
                                    start=True, stop=True)
                            for c2 in range(QW // 512):
                                c2s = slice(c2 * 512, (c2 + 1) * 512)
                                g2s = slice(qh * QW + c2 * 512,
                                            qh * QW + (c2 + 1) * 512)
                                nc.tensor.matmul(
                                    psB[:, c2s], lhsT=kT_s[ti][pr_B, ks],
                                    rhs=qT_s[ti][pr_B, g2s],
                                    start=True, stop=True)
                            # exp: one head on ScalarE, the other via the
                            # DVE bit-trick; swap per kt parity so the psum
                            # recycle chain alternates engines.
                            se_t = atp.tile([128, QW], b16, tag="ase",
                                            bufs=2)
                            dv_t = atp.tile([128, QW], i16, tag="adv",
                                            bufs=2)
                            if dve_exp:
                                ps_se, ps_dv = ((psA, psB) if kt % 2 == 0
                                                else (psB, psA))
                                nc.scalar.activation(se_t[:], ps_se[:],
                                                     func=Exp)
                                nc.vector.tensor_scalar(
                                    dv_t[:], ps_dv[:], SCH_A, SCH_B,
                                    MULT, ADD)
                                dv_b16 = dv_t[:].bitcast(b16)
                                at_cur = ((se_t[:], dv_b16) if kt % 2 == 0
                                          else (dv_b16, se_t[:]))
                            else:
                                nc.scalar.activation(se_t[:], psA[:],
                                                     func=Exp)
                                dvb = dv_t[:].bitcast(b16)
                                nc.scalar.activation(dvb, psB[:], func=Exp)
                                at_cur = (se_t[:], dvb)
                            if debug and ti == 0 and stage >= 5:
                                nc.sync.dma_start(
                                    attnd[ks, qs], at_cur[0])
                            if stage == 3:
                                sink(se_t)
                                sink(dv_t)
                                at_cur = None
                        if stage >= 4 and kt > 0 and at_prev is not None:
                            j = kt - 1
                            for c2 in range(QW // 512):
                                c2s = slice(c2 * 512, (c2 + 1) * 512)
                                nc.tensor.matmul(
                                    pvA[:, c2s], lhsT=vp_s[j][:, hA, :],
                                    rhs=at_prev[0][:, c2s],
                                    start=(j == 0), stop=(j == NKT - 1),
                                    skip_group_check=True)
                            for c2 in range(QW // 512):
                                c2s = slice(c2 * 512, (c2 + 1) * 512)
                                nc.tensor.matmul(
                                    pvB[:, c2s], lhsT=vp_s[j][:, hB, :],
                                    rhs=at_prev[1][:, c2s],
                                    start=(j == 0), stop=(j == NKT - 1),
                                    skip_group_check=True)
                        if kt < NKT:
                            at_prev = at_cur
                    if stage < 4:
                        return
                    # ---- drain: copy out, reciprocal of Z, normalize ----
                    scr = ti * NQH + qh
                    otA = dpool.tile([65, QW], f32, tag="ot", bufs=2)
                    nc.vector.tensor_copy(otA[:], pvA[:])
                    otB = dpool.tile([65, QW], f32, tag="ot", bufs=2)
                    nc.vector.tensor_copy(otB[:], pvB[:])
                    nc.sync.dma_start(
                        z_scr[scr][0:1, :, :].rearrange("a j p -> a (j p)"),
                        otA[64:65, :])
                    nc.sync.dma_start(
                        z_scr[scr][1:2, :, :].rearrange("a j p -> a (j p)"),
                        otB[64:65, :])
                    nj = QW // 128
                    zt = dpool.tile([128, 2 * nj], f32, tag="zt", bufs=2)
                    nc.sync.dma_start(
                        zt[:],
                        z_scr[scr][:, :, :].rearrange("a j p -> p (a j)"))
                    rzt = dpool.tile([128, 2 * nj], f32, tag="zt", bufs=2)
                    nc.vector.reciprocal(rzt[:], zt[:])
                    nc.sync.dma_start(
                        rz_scr[scr][:, :, :].rearrange("a j p -> p (a j)"),
                        rzt[:])
                    rzA = dpool.tile([1, QW], f32, tag="rz", bufs=2)
                    nc.sync.dma_start(
                        rzA[:],
                        rz_scr[scr][0:1, :, :].rearrange("a j p -> a (j p)"))
                    rzB = dpool.tile([1, QW], f32, tag="rz", bufs=2)
                    nc.sync.dma_start(
                        rzB[:],
                        rz_scr[scr][1:2, :, :].rearrange("a j p -> a (j p)"))
                    zbA = dpool.tile([64, QW], f32, tag="zb", bufs=2)
                    nc.gpsimd.partition_broadcast(zbA[:], rzA[:])
                    zbB = dpool.tile([64, QW], f32, tag="zb", bufs=2)
                    nc.gpsimd.partition_broadcast(zbB[:], rzB[:])
                    nc.vector.tensor_mul(aT_s[ti][pr_A, qs], otA[0:64, :],
                                         zbA[:])
                    nc.vector.tensor_mul(aT_s[ti][pr_B, qs], otB[0:64, :],
                                         zbB[:])

                if stage >= 3:
                    for ti in range(4):
                        for qh in range(NQH):
                            attn_pair(ti, qh)

                if stage == 4:
                    for t in aT_s:
                        sink(t)
                if debug and stage >= 3:
                    for o in range(4):
                        nc.sync.dma_start(qTd[o * 128:(o + 1) * 128, :],
                                          qT_s[o][:])
                        nc.sync.dma_start(kTd[o * 128:(o + 1) * 128, :],
                                          kT_s[o][:])
                if debug and stage >= 4:
                    for i in range(4):
                        nc.sync.dma_start(aTd[i * 128:(i + 1) * 128, :],
                                          aT_s[i][:])

            if stage < 5:
                fo = small.tile([128, 512], f32, tag="fout", bufs=2)
                nc.vector.tensor_copy(fo[:], sink_t[:])
                nc.sync.dma_start(outT[0:128, 0:512], fo[:])

            # ---- output projection ----
            if stage >= 5:
                wo_a = big.tile([128, 2, D], b16, tag="big", name="wo_a")
                wo_b = big.tile([128, 2, D], b16, tag="big", name="wo_b")
                for t in range(2):
                    nc.sync.dma_start(wo_a[:, t, :],
                                      woT[t * 128:(t + 1) * 128, :])
                    nc.sync.dma_start(wo_b[:, t, :],
                                      woT[(2 + t) * 128:(3 + t) * 128, :])
                wo_v = [wo_a[:, 0, :], wo_a[:, 1, :],
                        wo_b[:, 0, :], wo_b[:, 1, :]]
                with tc.tile_pool(name="ps2", bufs=1, space="PSUM") as psum2:
                    for o in range(8):
                        for c in range(NCH):
                            cs = slice(c * 512, (c + 1) * 512)
                            ps = psum2.tile([128, 512], f32, tag="fo",
                                            bufs=4)
                            for i in range(4):
                                nc.tensor.matmul(
                                    ps[:],
                                    lhsT=wo_v[i][:, o * 128:(o + 1) * 128],
                                    rhs=aT_s[i][:, cs],
                                    start=(i == 0), stop=(i == 3),
                                )
                            fo = small.tile([128, 512], f32, tag="fout",
                                            bufs=2)
                            nc.vector.tensor_copy(fo[:], ps[:])
                            nc.sync.dma_start(
                                outT[o * 128:(o + 1) * 128, cs], fo[:])

    nc.compile()
    return nc


def _host_prep(Q_in, K_in, V_in, W_q, W_k, W_v, W_o, s=S, reps=1,
               dve_exp=True):
    """Build per-core input maps (host-side shard + transpose + bf16 cast)."""
    in_maps = []
    scale = 1.0 / np.sqrt(np.float32(DK))
    for c in range(N_CORES):
        b, g = divmod(c, 2)
        gs = slice(g * G, (g + 1) * G)
        m = {
            "xqT": np.ascontiguousarray(Q_in[b].T).astype(bf16),
            "xkT": np.ascontiguousarray(K_in[b].T).astype(bf16),
            "xvT": np.ascontiguousarray(V_in[b].T).astype(bf16),
            "wqT": np.ascontiguousarray((W_q[gs, :] * scale).T).astype(bf16),
            "wkT": np.ascontiguousarray(W_k[gs, :].T).astype(bf16),
            "wvT": np.ascontiguousarray(W_v[gs, :].T).astype(bf16),
            "woT": np.ascontiguousarray(W_o[:, gs].T).astype(bf16),
            "vtag": np.zeros((5 + 8 * reps + (64 if dve_exp else 0),
                              _BUILD_VERSION), np.float32),
        }
        in_maps.append(m)
    return in_maps


def kernel(Q_in, K_in, V_in, W_q, W_k, W_v, W_o):
    from concourse.bass_utils import run_bass_kernel_spmd

    if "nc" not in _BUILD_CACHE:
        _BUILD_CACHE["nc"] = _build()
    nc = _BUILD_CACHE["nc"]

    in_maps = _host_prep(np.asarray(Q_in, np.float32), np.asarray(K_in, np.float32),
                         np.asarray(V_in, np.float32), np.asarray(W_q, np.float32),
                         np.asarray(W_k, np.float32), np.asarray(W_v, np.float32),
                         np.asarray(W_o, np.float32))
    res = run_bass_kernel_spmd(nc, in_maps, core_ids=list(range(N_CORES)))

    out = np.empty((B, S, D), np.float32)
    for b in range(B):
        acc = res.results[2 * b]["outT"] + res.results[2 * b + 1]["outT"]
        out[b] = acc.T
    return out
